# revision 2
# baseline (speedup 1.0000x reference)
"""Distributed Bass/Trainium2 kernel for the batch graph-Laplacian (k-NN)
loss.

Problem: z [8192, 512] fp32.  G = z z^T, d2_ij = ||z_i - z_j||^2, k=10
nearest neighbours per row (self excluded), W = max(A, A^T);
loss = (S_dir - 0.5*S_mut)/n over directed/mutual top-k edge squared
distances, so only the k-NN lists plus exact d2 on a small candidate set
are needed.

Device per core (rows of z sharded 1024/core, 8 row tiles of 128):
  - [1024, 8192] metric block nval_ij ~ G_ij - sq_j/2 + 256 via fp8e4m3
    DoubleRow matmuls (2 accumulating matmuls of effective K=256 per
    512-column PSUM bank; 2x bf16 throughput).  The column bias rides
    inside the matmul: z dims 510/511 are repurposed as a hi/lo fp8 pair
    (zq lanes = 1, zk lanes = fp8(256 - sq_j/2) + residual).
  - DoubleRow SBUF layout: contraction chunk c of 2 covers z dims
    [256c, 256c+256); dim 256c + 128*slot + p sits at partition p,
    slot-major flat column (slot*W + col).
  - Drain fused with a single 2:1 max reduction (keeps DVE/ACT well under
    the PE rate, which the deeper on-device tree used to exceed): ACT
    stages each even PSUM chunk to SBUF, DVE tensor_max's it against the
    odd chunk straight from PSUM into bf16 r1 [128, 4096]; r1 DMAs out.
    Reduced column u = 512g + j covers original columns {1024g + j,
    1024g + 512 + j}.
Host: top-TOPT positions per row (argpartition) over the [B, 4096]
  reduced array, each expands to its 2 covered columns; exact fp32 d2
  refinement, true top-10, mutual-edge resolution, scalar loss.
"""

import numpy as np
import ml_dtypes

B = 8192
D = 512
K = 10
N_CORES = 8
RPC = B // N_CORES          # rows per core = 1024
RT = RPC // 128             # row tiles per core = 8
RED = 2                     # reduction factor (single pairwise max)
RW = B // RED               # reduced width = 4096
TOPT = 48                   # host-side positions kept per row
SHIFT = 256.0
NVAL_BUFS = 2               # r1 tiles

_CACHE = {}


def _build_program(loop_iters=None, unroll=False):
    import concourse.bacc as bacc
    import concourse.mybir as mybir
    from concourse.tile import TileContext

    dt = mybir.dt
    nc = bacc.Bacc("TRN2", target_bir_lowering=False, debug=False,
                   num_devices=N_CORES)

    zq = nc.dram_tensor("zq", [2, 128, 2 * RPC], dt.float8e4,
                        kind="ExternalInput")
    zk = nc.dram_tensor("zk", [2, 128, 2 * B], dt.float8e4,
                        kind="ExternalInput")
    red_out = nc.dram_tensor("red", [RPC, RW], dt.bfloat16,
                             kind="ExternalOutput")

    with TileContext(nc) as tc:
        with (
            tc.tile_pool(name="const", bufs=1) as cpool,
            tc.tile_pool(name="nval", bufs=NVAL_BUFS) as npool,
            tc.tile_pool(name="psum", bufs=8, space="PSUM") as ppool,
            tc.tile_pool(name="stage", bufs=3) as spool,
        ):
            zk_sb = [cpool.tile([128, 2 * B], dt.float8e4, tag=f"zk{kc}",
                                name=f"zk_sb{kc}") for kc in range(2)]
            zq_sb = [cpool.tile([128, 2 * RPC], dt.float8e4, tag=f"zq{kc}",
                                name=f"zq_sb{kc}") for kc in range(2)]
            for kc in range(2):
                nc.sync.dma_start(zq_sb[kc][:], zq[kc, :, :])
            # both interleave slots' leading columns land first
            pieces = [slice(0, B // 2), slice(B, 3 * B // 2),
                      slice(B // 2, B), slice(3 * B // 2, 2 * B)]
            for sl in pieces:
                for kc in range(2):
                    nc.sync.dma_start(zk_sb[kc][:, sl], zk[kc, :, sl])

            from contextlib import nullcontext
            if loop_iters and unroll:
                for _ in range(loop_iters):
                    _body(nc, tc, npool, ppool, zq_sb, zk_sb, red_out, spool)
            else:
                loop_cm = (tc.For_i(0, loop_iters, 1) if loop_iters
                           else nullcontext())
                with loop_cm:
                    _body(nc, tc, npool, ppool, zq_sb, zk_sb, red_out, spool)

    nc.compile()
    return nc


def _body(nc, tc, npool, ppool, zq_sb, zk_sb, red_out, spool):
    import concourse.mybir as mybir
    dt = mybir.dt
    for m in range(RT):
        # r1 col u = 512g + j covers original cols 1024g + j, 1024g + 512 + j
        r1 = npool.tile([128, RW], dt.bfloat16, tag="r1")
        stage = None
        for g in range(16):
            ps = ppool.tile([128, 512], dt.float32, tag="ps", name="ps_t")
            csl = slice(g * 512, (g + 1) * 512)
            for kc in range(2):
                q3 = zq_sb[kc][:].rearrange("p (two m) -> p two m", two=2)
                k3 = zk_sb[kc][:].rearrange("p (two n) -> p two n", two=2)
                nc.tensor.matmul(
                    ps[:],
                    lhsT=q3[:, :, m * 128:(m + 1) * 128],
                    rhs=k3[:, :, csl],
                    start=(kc == 0),
                    stop=(kc == 1),
                    perf_mode=mybir.MatmulPerfMode.DoubleRow,
                )
            if g % 2 == 0:
                # DVE may read only ONE operand from PSUM: stage the even
                # chunk in SBUF via the scalar engine first
                stage = spool.tile([128, 512], dt.float32, tag="stage")
                nc.scalar.copy(stage[:], ps[:])
            else:
                rsl1 = slice((g // 2) * 512, (g // 2 + 1) * 512)
                nc.vector.tensor_max(r1[:, rsl1], stage[:], ps[:])
        rsl = slice(m * 128, (m + 1) * 128)
        nc.sync.dma_start(red_out[rsl, :], r1[:])


def _get_program():
    if "nc" not in _CACHE:
        _CACHE["nc"] = _build_program()
    return _CACHE["nc"]


def _prep_inputs(z):
    sq = np.einsum("ij,ij->i", z.astype(np.float64), z.astype(np.float64))
    bias = (SHIFT - 0.5 * sq).astype(np.float32)
    hi = bias.astype(ml_dtypes.float8_e4m3)
    lo = (bias - hi.astype(np.float32)).astype(ml_dtypes.float8_e4m3)
    zk_f = np.ascontiguousarray(z.T).astype(np.float32)
    zq_f = zk_f.copy()
    zk_f[510, :] = hi.astype(np.float32)
    zk_f[511, :] = lo.astype(np.float32)
    zq_f[510, :] = 1.0
    zq_f[511, :] = 1.0

    def to_dr(a, ncols):
        a8 = a.astype(ml_dtypes.float8_e4m3)
        a4 = a8.reshape(2, 2, 128, ncols)
        return np.ascontiguousarray(
            a4.transpose(0, 2, 1, 3)).reshape(2, 128, 2 * ncols)

    zk_dev = to_dr(zk_f, B)
    zq_devs = [to_dr(zq_f[:, c * RPC:(c + 1) * RPC], RPC)
               for c in range(N_CORES)]
    return sq, zk_dev, zq_devs


def kernel(z: np.ndarray) -> np.ndarray:
    from concourse.bass_utils import run_bass_kernel_spmd

    z = np.asarray(z, dtype=np.float32)
    assert z.shape == (B, D)

    sq, zk_dev, zq_devs = _prep_inputs(z)
    in_maps = [{"zq": zq_devs[c], "zk": zk_dev} for c in range(N_CORES)]

    nc = _get_program()
    res = run_bass_kernel_spmd(nc, in_maps, list(range(N_CORES)))
    _CACHE["last_result"] = res

    red = np.concatenate([res.results[c]["red"] for c in range(N_CORES)])
    return _postprocess(z, sq, red)


def _colmap():
    # replay the device reduction on index arrays: r1 col u = 512g + j
    # maxes original columns 1024g + j and 1024g + 512 + j
    u = np.arange(RW, dtype=np.int64)
    g, j = u // 512, u % 512
    return np.stack([1024 * g + j, 1024 * g + 512 + j], 1)    # [4096, 2]


_COLMAP = _colmap()


def _postprocess(z, sq, red):
    red32 = red.astype(np.float32)
    pos = np.argpartition(-red32, TOPT - 1, axis=1)[:, :TOPT]   # [B, T]
    cand_cols = _COLMAP[pos].reshape(B, TOPT * RED)             # [B, 96]
    rows = np.arange(B, dtype=np.int64)

    zc = z[cand_cols]
    dots = np.einsum("brd,bd->br", zc, z, optimize=True)
    d2 = sq[:, None] + sq[cand_cols] - 2.0 * dots.astype(np.float64)
    d2 = np.where(cand_cols == rows[:, None], np.inf, d2)

    sel = np.argpartition(d2, K - 1, axis=1)[:, :K]
    top_cols = np.take_along_axis(cand_cols, sel, axis=1)
    top_d2 = np.take_along_axis(d2, sel, axis=1)

    edge_key = rows[:, None] * B + top_cols
    rev_key = top_cols * B + rows[:, None]
    mutual = np.isin(rev_key, edge_key)

    s_dir = top_d2.sum()
    s_mut = top_d2[mutual].sum()
    loss = (s_dir - 0.5 * s_mut) / B
    return np.float32(loss)


# revision 4
# speedup vs baseline: 2.5282x; 2.5282x over previous
"""Distributed Bass/Trainium2 kernel for the batch graph-Laplacian (k-NN)
loss.

Problem: z [8192, 512] fp32.  G = z z^T, d2_ij = ||z_i - z_j||^2, k=10
nearest neighbours per row (self excluded), W = max(A, A^T);
loss = (S_dir - 0.5*S_mut)/n over directed/mutual top-k edge squared
distances, so only the k-NN lists plus exact d2 on a small candidate set
are needed.

Device per core (rows of z sharded 1024/core, 8 row tiles of 128):
  - [1024, 8192] metric block nval_ij ~ G_ij - sq_j/2 + 256 via fp8e4m3
    DoubleRowSwInterleave matmuls (HW-probed: the weight operand is read
    as 256 CONTIGUOUS sbuf columns from the AP base; buffer column c
    carries logical k-slot c%2 at logical output column 127 - c//2; the
    moving operand keeps the standard [p, 2, n] DoubleRow layout).
    SwInterleave measures ~197 ns/MM vs DoubleRow's ~235 (no HW weight
    deinterleave), a 16% tensor-engine win.  The column bias rides inside
    the matmul: z dims 510/511 are repurposed as a hi/lo fp8 pair
    (zq lanes = 1, zk lanes = fp8(256 - sq_j/2) + residual).
  - Drain in two-bank [128, 1024] PSUM tiles: ACT stages even tiles to
    SBUF, DVE tensor_max's them against odd tiles straight from PSUM,
    emitting fp8e4m3 r1 [128, 4096] (reduced col u covers original cols
    2048*(u//1024) + u%1024 and the same + 1024).  fp8 output halves HBM
    write volume: concurrent 8-core SBUF->HBM writes saturate at
    ~90 GB/s/core, so the 4 MB/core result is what keeps DMA (~45us)
    under the PE roofline (~50us).
Host: top-TOPT positions per row (argpartition) over the [B, 4096]
  reduced array, each expands to its 2 covered columns; exact fp32 d2
  refinement, true top-10, mutual-edge resolution, scalar loss.
"""

import numpy as np
import ml_dtypes

B = 8192
D = 512
K = 10
N_CORES = 8
RPC = B // N_CORES          # rows per core = 1024
RT = RPC // 128             # row tiles per core = 8
RED = 2                     # reduction factor (single pairwise max)
RW = B // RED               # reduced width = 4096
TOPT = 96                   # host-side positions kept per row
SHIFT = 256.0
NVAL_BUFS = 2               # r1 tiles

_CACHE = {}


def _build_program(loop_iters=None, unroll=False):
    import concourse.bacc as bacc
    import concourse.mybir as mybir
    from concourse.tile import TileContext

    dt = mybir.dt
    nc = bacc.Bacc("TRN2", target_bir_lowering=False, debug=False,
                   num_devices=N_CORES)

    zq = nc.dram_tensor("zq", [2, 128, 2 * RPC], dt.float8e4,
                        kind="ExternalInput")
    zk = nc.dram_tensor("zk", [2, 128, 2 * B], dt.float8e4,
                        kind="ExternalInput")
    red_out = nc.dram_tensor("red", [RPC, RW], dt.float8e4,
                             kind="ExternalOutput")

    with TileContext(nc) as tc:
        with (
            tc.tile_pool(name="const", bufs=1) as cpool,
            tc.tile_pool(name="nval", bufs=NVAL_BUFS) as npool,
            tc.tile_pool(name="psum", bufs=4, space="PSUM") as ppool,
            tc.tile_pool(name="stage", bufs=3) as spool,
        ):
            zk_sb = [cpool.tile([128, 2 * B], dt.float8e4, tag=f"zk{kc}",
                                name=f"zk_sb{kc}") for kc in range(2)]
            zq_sb = [cpool.tile([128, 2 * RPC], dt.float8e4, tag=f"zq{kc}",
                                name=f"zq_sb{kc}") for kc in range(2)]
            for kc in range(2):
                nc.sync.dma_start(zq_sb[kc][:], zq[kc, :, :])
            # both interleave slots' leading columns land first
            pieces = [slice(0, B // 2), slice(B, 3 * B // 2),
                      slice(B // 2, B), slice(3 * B // 2, 2 * B)]
            for sl in pieces:
                for kc in range(2):
                    nc.sync.dma_start(zk_sb[kc][:, sl], zk[kc, :, sl])

            from contextlib import nullcontext
            if loop_iters and unroll:
                for _ in range(loop_iters):
                    _body(nc, tc, npool, ppool, zq_sb, zk_sb, red_out, spool)
            else:
                loop_cm = (tc.For_i(0, loop_iters, 1) if loop_iters
                           else nullcontext())
                with loop_cm:
                    _body(nc, tc, npool, ppool, zq_sb, zk_sb, red_out, spool)

    nc.compile()
    return nc


def _body(nc, tc, npool, ppool, zq_sb, zk_sb, red_out, spool):
    import concourse.mybir as mybir
    dt = mybir.dt
    for m in range(RT):
        # two-bank tiles t cover chunks (2t, 2t+1); drain pairs (T2p, T2p+1)
        # so r1 col 1024p + c = max(original col 2048p + c, 2048p + 1024 + c)
        r1 = npool.tile([128, RW], dt.float8e4, tag="r1")
        stage = None
        for t in range(8):
            ps = ppool.tile([128, 1024], dt.float32, tag="ps", name="ps_t")
            for c in range(2):
                g = 2 * t + c
                csl = slice(g * 512, (g + 1) * 512)
                for kc in range(2):
                    # SWI weights: 256 contiguous columns per m-tile
                    q3 = zq_sb[kc][:, m * 256:(m + 1) * 256].rearrange(
                        "p (two mm) -> p two mm", two=2)
                    k3 = zk_sb[kc][:].rearrange("p (two n) -> p two n", two=2)
                    nc.tensor.matmul(
                        ps[:, c * 512:(c + 1) * 512],
                        lhsT=q3,
                        rhs=k3[:, :, csl],
                        start=(kc == 0),
                        stop=(kc == 1),
                        perf_mode=mybir.MatmulPerfMode.DoubleRowSwInterleave,
                    )
            if t % 2 == 0:
                # DVE may read only ONE operand from PSUM: stage the even
                # tile in SBUF via the scalar engine first
                stage = spool.tile([128, 1024], dt.float32, tag="stage")
                nc.scalar.copy(stage[:], ps[:])
            else:
                rsl1 = slice((t // 2) * 1024, (t // 2 + 1) * 1024)
                nc.vector.tensor_max(r1[:, rsl1], stage[:], ps[:])
        rsl = slice(m * 128, (m + 1) * 128)
        nc.sync.dma_start(red_out[rsl, :], r1[:])


def _get_program():
    if "nc" not in _CACHE:
        _CACHE["nc"] = _build_program()
    return _CACHE["nc"]


def _to_swi(a, ncols):
    """Weights packing for DoubleRowSwInterleave (HW-probed semantics).

    a: [512, ncols] fp32.  Per 128-col m-tile, a contiguous 256-column
    region: buffer column c holds k-slot c%2 of logical column 127 - c//2.
    k-slot s of contraction chunk kc covers z dims 256kc + 128s + p.
    """
    a8 = a.astype(ml_dtypes.float8_e4m3)
    a4 = a8.reshape(2, 2, 128, ncols)           # [kc, slot, p, n]
    c = np.arange(2 * ncols)
    mtile = c // 256
    cc = c % 256
    slot_req = cc % 2
    col_req = mtile * 128 + 127 - cc // 2
    blk = a4[:, slot_req, :, col_req]           # [2*ncols, kc, 128]
    return np.ascontiguousarray(blk.transpose(1, 2, 0))


def _to_dr(a, ncols):
    """Moving-operand packing (standard DoubleRow interleave layout)."""
    a8 = a.astype(ml_dtypes.float8_e4m3)
    a4 = a8.reshape(2, 2, 128, ncols)
    return np.ascontiguousarray(
        a4.transpose(0, 2, 1, 3)).reshape(2, 128, 2 * ncols)


def _prep_inputs(z):
    sq = np.einsum("ij,ij->i", z.astype(np.float64), z.astype(np.float64))
    bias = (SHIFT - 0.5 * sq).astype(np.float32)
    hi = bias.astype(ml_dtypes.float8_e4m3)
    lo = (bias - hi.astype(np.float32)).astype(ml_dtypes.float8_e4m3)
    zk_f = np.ascontiguousarray(z.T).astype(np.float32)
    zq_f = zk_f.copy()
    zk_f[510, :] = hi.astype(np.float32)
    zk_f[511, :] = lo.astype(np.float32)
    zq_f[510, :] = 1.0
    zq_f[511, :] = 1.0

    zk_dev = _to_dr(zk_f, B)
    zq_devs = [_to_swi(zq_f[:, c * RPC:(c + 1) * RPC], RPC)
               for c in range(N_CORES)]
    return sq, zk_dev, zq_devs


def kernel(z: np.ndarray) -> np.ndarray:
    from concourse.bass_utils import run_bass_kernel_spmd

    z = np.asarray(z, dtype=np.float32)
    assert z.shape == (B, D)

    sq, zk_dev, zq_devs = _prep_inputs(z)
    in_maps = [{"zq": zq_devs[c], "zk": zk_dev} for c in range(N_CORES)]

    nc = _get_program()
    res = run_bass_kernel_spmd(nc, in_maps, list(range(N_CORES)))
    _CACHE["last_result"] = res

    red = np.concatenate([res.results[c]["red"] for c in range(N_CORES)])
    return _postprocess(z, sq, red)


def _colmap():
    # replay the device reduction: r1 col u = 1024p + c covers original
    # columns 2048p + c and 2048p + 1024 + c
    u = np.arange(RW, dtype=np.int64)
    p, c = u // 1024, u % 1024
    return np.stack([2048 * p + c, 2048 * p + 1024 + c], 1)   # [4096, 2]


_COLMAP = _colmap()


def _postprocess(z, sq, red):
    red32 = np.asarray(red).astype(np.float32)
    pos = np.argpartition(-red32, TOPT - 1, axis=1)[:, :TOPT]   # [B, T]
    cand_cols = _COLMAP[pos].reshape(B, TOPT * RED)             # [B, 192]
    rows = np.arange(B, dtype=np.int64)

    zc = z[cand_cols]
    dots = np.einsum("brd,bd->br", zc, z, optimize=True)
    d2 = sq[:, None] + sq[cand_cols] - 2.0 * dots.astype(np.float64)
    d2 = np.where(cand_cols == rows[:, None], np.inf, d2)

    sel = np.argpartition(d2, K - 1, axis=1)[:, :K]
    top_cols = np.take_along_axis(cand_cols, sel, axis=1)
    top_d2 = np.take_along_axis(d2, sel, axis=1)

    edge_key = rows[:, None] * B + top_cols
    rev_key = top_cols * B + rows[:, None]
    mutual = np.isin(rev_key, edge_key)

    s_dir = top_d2.sum()
    s_mut = top_d2[mutual].sum()
    loss = (s_dir - 0.5 * s_mut) / B
    return np.float32(loss)
